# revision 42
# baseline (speedup 1.0000x reference)
"""Trainium2 Bass kernel for nn_AttentionBlock (sliding-window GQA, gpt-oss style).

Sharding: pure data-parallel over tokens. B=4 batches x 2 T-halves of 512
tokens = 8 shards, one per NeuronCore; the 128-token sliding window means each
shard only needs a 128-token K/V halo, so there is no cross-core traffic.

v2 (this file) vs the v1 baseline:
  * every matmul operand is bf16 (f32 PSUM accumulate) -> half the weight DMA
    and full-rate PE everywhere, incl. the small attention tiles.
  * attention uses 128-query x two-128-key triangle blocks (prev + diag).
    Per (head-pair, 128-query block) the scores psum is [128k, (kb, h, q)] and
    the visible set is exactly the two complementary triangles, applied as a
    multiplicative bf16 0/1 mask on the exp'd tile (DVE), so no PE cycles are
    burned on additive mask matmuls.
  * PV runs per head with tile_position=(0,0)/(0,64) so the two heads' O.T
    land stacked in one [128=(2h x 64vf), 128q] psum tile -- which is exactly
    the lhsT layout the out-projection wants; the normalize multiply is the
    evacuation (one DVE op per unit).
  * softmax denominators: ones-column matmuls into a [1, (h,q)] psum row,
    DMA-gathered 4 rows at a time into a per-head-pair [8, 128] tile, one
    reciprocal_approx_fast per head-pair, then broadcast back to 64 vf
    partitions with a tiny selector matmul ([8,128] x [8,128] -> [128,128]).
  * per-head attention sinks are folded in as a per-partition tensor_scalar
    add on the packed denominator tile.

On-chip layouts (per core):
  xT        [dmodel, 640tok] bf16 (halo 128 + 512 own; halo zero-padded on
                                   the first half of each sequence)
  K.T       [64, 640] bf16 per kv head (rotated)
  Q.T       [64, 2, 512] bf16 per head pair (rotated)
  V         [128tok, 8kv, 64] bf16 per 128-token chunk (direct [tok, feat]
            projection: lhsT = xT tile, rhs = wv.T tile -- no PE transpose)
  scores    S.T [128key, (kb, h, 128q)] psum; exp on scalar engine; 0/1
            triangle mask on DVE
  out proj  Y [tok, dmodel] accumulated over 16 feature chunks + K=1 bias
            matmul, evacuated f32
"""

import math
from contextlib import ExitStack

import numpy as np

import concourse.bacc as bacc
import concourse.tile as tile
from concourse import mybir
from concourse.bass_utils import run_bass_kernel_spmd

_DEBUG = False
F32 = mybir.dt.float32
BF16 = mybir.dt.bfloat16
AF = mybir.ActivationFunctionType
ALU = mybir.AluOpType

B, T, D = 4, 1024, 2048
HEAD_DIM = 64
N_HEADS = 32
N_KV = 8
WINDOW = 128
SM_SCALE = 1.0 / math.sqrt(HEAD_DIM)
ROPE_THETA = 150000.0
SCALING = 32.0
NTK_ALPHA = 1.0
NTK_BETA = 32.0
ICL = 1024

TQ = 512          # queries per shard
HALO = 128
TOK = TQ + HALO   # 640 tokens of K/V context per shard
NKT = D // 128    # 16 contraction tiles over dmodel
NQF = 16          # Q feature tiles (2048 features = 16 head pairs)
NKF = 4           # K feature tiles (512 features = 4 kv-head pairs)
NVC = TOK // 128  # 5 V token chunks
NQB = TQ // 128   # 4 query blocks per shard


def _build_nc():
    nc = bacc.Bacc("TRN2", target_bir_lowering=False, debug=False)

    # ---- DRAM I/O ----
    # xt is partition-major: one DMA, 128 contiguous ~20KB descriptors
    xt = nc.dram_tensor("xt", (128, NKT, TOK), BF16, kind="ExternalInput")
    wq = nc.dram_tensor("wq", (NQF, 128, D), BF16, kind="ExternalInput")
    wk = nc.dram_tensor("wk", (NKF, 128, D), BF16, kind="ExternalInput")
    wv = nc.dram_tensor("wv", (4, 128, D), BF16, kind="ExternalInput")
    wo = nc.dram_tensor("wo", (16, 128, D), BF16, kind="ExternalInput")
    qbias = nc.dram_tensor("qbias", (128, NQF), F32, kind="ExternalInput")
    kbias = nc.dram_tensor("kbias", (128, NKF), F32, kind="ExternalInput")
    vbias = nc.dram_tensor("vbias", (1, 512), BF16, kind="ExternalInput")
    outb = nc.dram_tensor("outb", (1, D), BF16, kind="ExternalInput")
    # rope tables: cos duplicated per head; sinA/sinB are the half-shifted
    # signed sin tables so the rotate-half multiply needs no partition swap
    cosq = nc.dram_tensor("cosq", (128, TQ), BF16, kind="ExternalInput")
    sinqa = nc.dram_tensor("sinqa", (128, TQ), BF16, kind="ExternalInput")
    sinqb = nc.dram_tensor("sinqb", (128, TQ), BF16, kind="ExternalInput")
    cosk = nc.dram_tensor("cosk", (128, TOK), BF16, kind="ExternalInput")
    sinka = nc.dram_tensor("sinka", (128, TOK), BF16, kind="ExternalInput")
    sinkb = nc.dram_tensor("sinkb", (128, TOK), BF16, kind="ExternalInput")
    maskt = nc.dram_tensor("maskt", (128, 512), BF16, kind="ExternalInput")
    mask0 = nc.dram_tensor("mask0", (128, 512), BF16, kind="ExternalInput")
    esinkc = nc.dram_tensor("esinkc", (8, NQF), F32, kind="ExternalInput")
    selqb = nc.dram_tensor("selqb", (8, NQB, 128), BF16, kind="ExternalInput")
    ones128 = nc.dram_tensor("ones128", (128, 1), BF16, kind="ExternalInput")
    onescol = nc.dram_tensor("onescol", (1, 128), BF16, kind="ExternalInput")
    y = nc.dram_tensor("y", (TQ, D), F32, kind="ExternalOutput")
    if _DEBUG:
        dbg_k = nc.dram_tensor("dbg_k", (64, TOK), BF16, kind="ExternalOutput")
        dbg_q = nc.dram_tensor("dbg_q", (64, 2, TQ), BF16, kind="ExternalOutput")
        dbg_v = nc.dram_tensor("dbg_v", (128, 64), BF16, kind="ExternalOutput")
        dbg_pt = nc.dram_tensor("dbg_pt", (128, 512), BF16, kind="ExternalOutput")
        dbg_dn = nc.dram_tensor("dbg_dn", (8, 128), F32, kind="ExternalOutput")
        dbg_opk = nc.dram_tensor("dbg_opk", (128, 128), BF16, kind="ExternalOutput")

    with tile.TileContext(nc) as tc, ExitStack() as ctx:
        ep = ctx.enter_context
        const = ep(tc.tile_pool(name="const", bufs=1))
        wqp = ep(tc.tile_pool(name="wqp", bufs=4))
        kthp = ep(tc.tile_pool(name="kthp", bufs=8))    # rotated K per kv head
        vsbp = ep(tc.tile_pool(name="vsbp", bufs=NVC))  # V chunks, held
        qhdp = ep(tc.tile_pool(name="qhdp", bufs=3))
        qrbp = ep(tc.tile_pool(name="qrbp", bufs=3))
        ptbep = ep(tc.tile_pool(name="ptbep", bufs=2))
        ptbp = ep(tc.tile_pool(name="ptbp", bufs=4))
        prp = ep(tc.tile_pool(name="prp", bufs=2))
        pwp = ep(tc.tile_pool(name="pwp", bufs=2))
        opknp = ep(tc.tile_pool(name="opknp", bufs=NQF * NQB))  # held for out proj
        drpp = ep(tc.tile_pool(name="drpp", bufs=2))
        dnsp = ep(tc.tile_pool(name="dnsp", bufs=2))
        rpssp = ep(tc.tile_pool(name="rpssp", bufs=2))
        drsp = ep(tc.tile_pool(name="drsp", bufs=2))
        drrp = ep(tc.tile_pool(name="drrp", bufs=2))
        drbp = ep(tc.tile_pool(name="drbp", bufs=2))
        xtp = ep(tc.tile_pool(name="xtp", bufs=4))
        # K/V-phase-only pools: created last (top of the SBUF pool stack) so
        # they can be released before the wo prefetch reuses their space
        kvctx = ExitStack()
        wkp = kvctx.enter_context(tc.tile_pool(name="wkp", bufs=4))
        wvp = kvctx.enter_context(tc.tile_pool(name="wvp", bufs=4))
        khdp = kvctx.enter_context(tc.tile_pool(name="khdp", bufs=2))
        if True:
            # ---- highest-priority input DMAs first, spread over queues:
            # xt as 4 parallel transfers (2 on gpsimd, 2 on vector), K weights
            # ahead of Q weights on sync so the K phase is never starved
            xtq = []
            for g in range(4):
                t = xtp.tile([128, 4, TOK], BF16, tag="xt", name=f"xt{g}")
                eng = nc.gpsimd if g % 2 == 0 else nc.scalar
                eng.dma_start(out=t, in_=xt[:, 4 * g : 4 * g + 4, :])
                xtq.append(t)
            xts = [xtq[kt // 4][:, kt % 4, :] for kt in range(NKT)]
            wks = []
            for fk in range(NKF):
                t = wkp.tile([128, NKT, 128], BF16, tag="wk")
                nc.sync.dma_start(out=t, in_=wk[fk])
                wks.append(t)
            wvs = []
            for g in range(4):
                t = wvp.tile([128, 4, 512], BF16, tag="wv")
                nc.gpsimd.dma_start(out=t, in_=wv[g].rearrange("p (g f) -> p g f", g=4))
                wvs.append(t)
            # first few Q weight tiles on sync (rest just-in-time in the loop)
            wqs = [None] * NQF
            for qf in range(4):
                wqs[qf] = wqp.tile([128, NKT, 128], BF16, tag="wq", name=f"wq{qf}")
                nc.sync.dma_start(out=wqs[qf], in_=wq[qf])

            # ---- constants (scalar queue, K-phase needs first) ----
            kbias_sb = const.tile([128, NKF], F32)
            nc.scalar.dma_start(out=kbias_sb, in_=kbias[:])
            cosk_sb = const.tile([128, TOK], BF16)
            nc.scalar.dma_start(out=cosk_sb, in_=cosk[:])
            sinka_sb = const.tile([128, TOK], BF16)
            nc.scalar.dma_start(out=sinka_sb, in_=sinka[:])
            sinkb_sb = const.tile([128, TOK], BF16)
            nc.scalar.dma_start(out=sinkb_sb, in_=sinkb[:])
            vbias_sb = const.tile([1, 512], BF16)
            nc.scalar.dma_start(out=vbias_sb, in_=vbias[:])
            onescol_sb = const.tile([1, 128], BF16)
            nc.scalar.dma_start(out=onescol_sb, in_=onescol[:])
            qbias_sb = const.tile([128, NQF], F32)
            nc.scalar.dma_start(out=qbias_sb, in_=qbias[:])
            cosq_sb = const.tile([128, TQ], BF16)
            nc.scalar.dma_start(out=cosq_sb, in_=cosq[:])
            sinqa_sb = const.tile([128, TQ], BF16)
            nc.scalar.dma_start(out=sinqa_sb, in_=sinqa[:])
            sinqb_sb = const.tile([128, TQ], BF16)
            nc.scalar.dma_start(out=sinqb_sb, in_=sinqb[:])
            maskt_sb = const.tile([128, 512], BF16)
            nc.scalar.dma_start(out=maskt_sb, in_=maskt[:])
            mask0_sb = const.tile([128, 512], BF16)
            nc.scalar.dma_start(out=mask0_sb, in_=mask0[:])
            esinkc_sb = const.tile([8, NQF], F32)
            nc.scalar.dma_start(out=esinkc_sb, in_=esinkc[:])
            selqb_sb = const.tile([8, NQB, 128], BF16)
            nc.scalar.dma_start(out=selqb_sb, in_=selqb[:])
            ones128_sb = const.tile([128, 1], BF16)
            nc.scalar.dma_start(out=ones128_sb, in_=ones128[:])
            outb_sb = const.tile([1, D], BF16)
            nc.scalar.dma_start(out=outb_sb, in_=outb[:])

            pctx = ExitStack()
            pspj = pctx.enter_context(tc.tile_pool(name="pspj", bufs=2, space="PSUM"))
            psst = pctx.enter_context(tc.tile_pool(name="psst", bufs=2, space="PSUM"))
            psop = pctx.enter_context(tc.tile_pool(name="psop", bufs=2, space="PSUM"))
            psrp = pctx.enter_context(tc.tile_pool(name="psrp", bufs=1, space="PSUM"))
            psdn = pctx.enter_context(tc.tile_pool(name="psdn", bufs=1, space="PSUM"))
            if True:
                def rope_pair(dsts, src, cos_t, sina_t, sinb_t, n):
                    """Rotate both 64-row heads of src [128, n] into dsts[h]
                    [64, n] without any partition swap: the rotate-half
                    product uses half-shifted sin tables so every
                    tensor_tensor's two inputs share a base partition."""
                    pr = prp.tile([128, TOK], BF16, tag="ropepr")
                    nc.vector.tensor_mul(
                        out=pr[:, :n], in0=src[:, :n], in1=cos_t[:, :n]
                    )
                    pw = pwp.tile([128, TOK], BF16, tag="ropepw")
                    for b0 in (0, 64):
                        nc.vector.tensor_mul(
                            out=pw[b0 : b0 + 32, :n],
                            in0=src[b0 + 32 : b0 + 64, :n],
                            in1=sina_t[b0 + 32 : b0 + 64, :n],
                        )
                        nc.vector.tensor_mul(
                            out=pw[b0 + 32 : b0 + 64, :n],
                            in0=src[b0 : b0 + 32, :n],
                            in1=sinb_t[b0 : b0 + 32, :n],
                        )
                    for h2 in range(2):
                        nc.vector.tensor_add(
                            out=dsts[h2],
                            in0=pr[64 * h2 : 64 * h2 + 64, :n],
                            in1=pw[64 * h2 : 64 * h2 + 64, :n],
                        )

                # ---- K projection + rope: 8 kv heads as 4 pairs ----
                kth = [None] * N_KV
                for fk in range(NKF):
                    wsb = wks[fk]
                    khd = khdp.tile([128, TOK], BF16, tag="khd")
                    for c0 in (0, 320):
                        pst = pspj.tile([128, 512], F32, tag="pj")
                        for k in range(NKT):
                            nc.tensor.matmul(
                                pst[:, 0:320],
                                wsb[:, k, :],
                                xts[k][:, c0 : c0 + 320],
                                start=(k == 0),
                                stop=(k == NKT - 1),
                            )
                        nc.scalar.activation(
                            out=khd[:, c0 : c0 + 320],
                            in_=pst[:, 0:320],
                            func=AF.Identity,
                            bias=kbias_sb[:, fk : fk + 1],
                        )
                    kt0 = kthp.tile([64, TOK], BF16, tag="kth")
                    kt1 = kthp.tile([64, TOK], BF16, tag="kth")
                    rope_pair([kt0, kt1], khd, cosk_sb, sinka_sb, sinkb_sb, TOK)
                    kth[2 * fk] = kt0
                    kth[2 * fk + 1] = kt1
                if _DEBUG:
                    nc.sync.dma_start(out=dbg_k[:], in_=kth[0])

                # ---- V: direct [tok, feat] projection ----
                vsbs = []
                for c in range(NVC):
                    psv = pspj.tile([128, 512], F32, tag="pj")
                    for k in range(NKT):
                        nc.tensor.matmul(
                            psv,
                            xts[k][:, 128 * c : 128 * c + 128],
                            wvs[k // 4][:, k % 4, :],
                            start=(k == 0),
                            stop=False,
                        )
                    nc.tensor.matmul(
                        psv, onescol_sb, vbias_sb, start=False, stop=True
                    )
                    vsb = vsbp.tile([128, N_KV, 64], BF16, tag="v")
                    nc.scalar.activation(
                        out=vsb,
                        in_=psv.rearrange("p (h d) -> p h d", h=N_KV),
                        func=AF.Copy,
                    )
                    vsbs.append(vsb)
                if _DEBUG:
                    nc.sync.dma_start(out=dbg_v[:], in_=vsbs[2][:, 0, :])

                # ---- out-proj weight prefetch (first half) ----
                kvctx.close()  # free wk/wv/khd SBUF for the wo tiles
                wop = ctx.enter_context(tc.tile_pool(name="wop", bufs=32))
                wos = []
                for ft in range(16):
                    wosb = wop.tile([128, 1024], BF16, tag="wo")
                    nc.gpsimd.dma_start(out=wosb, in_=wo[ft][:, 0:1024])
                    wos.append(wosb)

                # ---- Q projection + attention per head pair ----
                opkns = [[None] * NQB for _ in range(NQF)]
                for qf in range(NQF):
                    h_kv = qf // 2
                    if qf + 4 < NQF:
                        wqs[qf + 4] = wqp.tile([128, NKT, 128], BF16, tag="wq", name=f"wq{qf+4}")
                        nc.sync.dma_start(out=wqs[qf + 4], in_=wq[qf + 4])
                    if qf == 11:
                        # prefetch second half of out-proj weights
                        for ft in range(16):
                            wosb = wop.tile([128, 1024], BF16, tag="wo")
                            nc.gpsimd.dma_start(
                                out=wosb, in_=wo[ft][:, 1024:2048]
                            )
                            wos.append(wosb)
                    wsb = wqs[qf]
                    psq = pspj.tile([128, 512], F32, tag="pj")
                    for k in range(NKT):
                        nc.tensor.matmul(
                            psq,
                            wsb[:, k, :],
                            xts[k][:, HALO:TOK],
                            start=(k == 0),
                            stop=(k == NKT - 1),
                        )
                    qhd = qhdp.tile([128, TQ], BF16, tag="qhd")
                    nc.scalar.activation(
                        out=qhd, in_=psq, func=AF.Identity,
                        bias=qbias_sb[:, qf : qf + 1],
                    )
                    qrb = qrbp.tile([64, 2, TQ], BF16, tag="qrb")
                    rope_pair(
                        [qrb[:, 0, :], qrb[:, 1, :]], qhd,
                        cosq_sb, sinqa_sb, sinqb_sb, TQ,
                    )
                    if _DEBUG and qf == 0:
                        nc.sync.dma_start(out=dbg_q[:], in_=qrb)

                    dn = psdn.tile([1, 512], F32, tag="dn")
                    drp = drpp.tile([8, 128], F32, tag="drp")
                    ops4 = psop.tile([128, NQB, 128], F32, tag="ops")
                    for qb in range(NQB):
                        # scores: prev + diag key blocks
                        st = psst.tile([128, 512], F32, tag="st")
                        qs = qrb[:, :, 128 * qb : 128 * qb + 128]
                        nc.tensor.matmul(
                            st[:, 0:256],
                            kth[h_kv][:, 128 * qb : 128 * qb + 128],
                            qs, start=True, stop=True,
                        )
                        nc.tensor.matmul(
                            st[:, 256:512],
                            kth[h_kv][:, 128 * qb + 128 : 128 * qb + 256],
                            qs, start=True, stop=True,
                        )
                        ptbe = ptbep.tile([128, 512], BF16, tag="ptbe")
                        nc.scalar.activation(
                            out=ptbe, in_=st, func=AF.Exp, scale=SM_SCALE
                        )
                        ptb = ptbp.tile([128, 512], BF16, tag="ptb")
                        nc.vector.tensor_mul(
                            out=ptb, in0=ptbe,
                            in1=(mask0_sb if qb == 0 else maskt_sb),
                        )
                        if _DEBUG and qf == 0 and qb == 1:
                            nc.sync.dma_start(out=dbg_pt[:], in_=ptb)
                        # denominator rows: [1, (h, q)] accumulated over kb
                        dslot = dn[0:1, 256 * (qb % 2) : 256 * (qb % 2) + 256]
                        nc.tensor.matmul(
                            dslot, ones128_sb, ptb[:, 0:256], start=True, stop=False
                        )
                        nc.tensor.matmul(
                            dslot, ones128_sb, ptb[:, 256:512], start=False, stop=True
                        )
                        # PV: per head, stacked halves of one [128, 128] slot
                        for h2 in range(2):
                            nc.tensor.matmul(
                                ops4[64 * h2 : 64 * h2 + 64, qb, :],
                                vsbs[qb][:, h_kv, :],
                                ptb[:, 128 * h2 : 128 * h2 + 128],
                                start=True, stop=False,
                                tile_position=(0, 64 * h2),
                            )
                            nc.tensor.matmul(
                                ops4[64 * h2 : 64 * h2 + 64, qb, :],
                                vsbs[qb + 1][:, h_kv, :],
                                ptb[:, 256 + 128 * h2 : 256 + 128 * h2 + 128],
                                start=False, stop=True,
                                tile_position=(0, 64 * h2),
                            )
                        if qb % 2 == 1:
                            # evac both units' denom rows, gather -> [4, 128]
                            dn_sb = dnsp.tile([1, 512], F32, tag="dnsb")
                            nc.scalar.activation(out=dn_sb, in_=dn, func=AF.Copy)
                            nc.gpsimd.dma_start(
                                out=drp[2 * (qb - 1) : 2 * (qb - 1) + 4, :],
                                in_=dn_sb,
                            )
                    # packed sink-add + reciprocal for this head pair
                    drs = drsp.tile([8, 128], F32, tag="drs")
                    nc.vector.tensor_scalar(
                        out=drs, in0=drp,
                        scalar1=esinkc_sb[:, qf : qf + 1], scalar2=None,
                        op0=ALU.add,
                    )
                    drr = drrp.tile([8, 128], F32, tag="drr")
                    nc.vector.reciprocal_approx_fast(out=drr, in_=drs)
                    drb = drbp.tile([8, 128], BF16, tag="drb")
                    nc.vector.tensor_copy(out=drb, in_=drr)
                    if _DEBUG and qf == 0:
                        nc.sync.dma_start(out=dbg_dn[:], in_=drr)
                    rps4 = psrp.tile([128, NQB, 128], F32, tag="rps")
                    for qb in range(NQB):
                        nc.tensor.matmul(
                            rps4[:, qb, :], selqb_sb[:, qb, :], drb,
                            start=True, stop=True,
                        )
                    # DVE may read only one PSUM input: evac rps first
                    rps_sb = rpssp.tile([128, NQB, 128], BF16, tag="rpssb")
                    nc.scalar.activation(out=rps_sb, in_=rps4, func=AF.Copy)
                    for qb in range(NQB):
                        opkn = opknp.tile([128, 128], BF16, tag="opkn")
                        nc.vector.tensor_mul(
                            out=opkn, in0=ops4[:, qb, :], in1=rps_sb[:, qb, :]
                        )
                        opkns[qf][qb] = opkn
                        if _DEBUG and qf == 0 and qb == 1:
                            nc.sync.dma_start(out=dbg_opk[:], in_=opkn)

            # ---- output projection ----
            pctx.close()
            yp = ctx.enter_context(tc.tile_pool(name="yp", bufs=2))
            psy = ctx.enter_context(tc.tile_pool(name="psy", bufs=2, space="PSUM"))
            if True:
                for chp in range(2):
                    for qb in range(NQB):
                        for c2 in range(2):
                            ch = chp * 2 + c2
                            yps = psy.tile([128, 512], F32, tag="psy")
                            for ft in range(16):
                                nc.tensor.matmul(
                                    yps,
                                    opkns[ft][qb],
                                    wos[chp * 16 + ft][:, c2 * 512 : c2 * 512 + 512],
                                    start=(ft == 0),
                                    stop=False,
                                )
                            nc.tensor.matmul(
                                yps,
                                onescol_sb,
                                outb_sb[:, ch * 512 : ch * 512 + 512],
                                start=False, stop=True,
                            )
                            ysb = yp.tile([128, 512], F32, tag="y")
                            nc.scalar.activation(out=ysb, in_=yps, func=AF.Copy)
                            nc.sync.dma_start(
                                out=y[qb * 128 : (qb + 1) * 128, ch * 512 : ch * 512 + 512],
                                in_=ysb,
                            )

    nc.compile()
    return nc


_NC_CACHE = None


def _get_nc():
    global _NC_CACHE
    if _NC_CACHE is None:
        _NC_CACHE = _build_nc()
    return _NC_CACHE


def _rope_tables(positions):
    """fp32 YaRN/NTK-by-parts tables, matching the reference bit-for-bit."""
    d_half = HEAD_DIM // 2
    freq = ROPE_THETA ** (np.arange(0, HEAD_DIM, 2, dtype=np.float32) / HEAD_DIM)
    concentration = 0.1 * math.log(SCALING) + 1.0
    low = d_half * math.log(ICL / (NTK_BETA * 2 * math.pi)) / math.log(ROPE_THETA)
    high = d_half * math.log(ICL / (NTK_ALPHA * 2 * math.pi)) / math.log(ROPE_THETA)
    interpolation = 1.0 / (SCALING * freq)
    extrapolation = 1.0 / freq
    ramp = np.clip(
        (np.arange(d_half, dtype=np.float32) - low) / (high - low), 0.0, 1.0
    )
    inv_freq = interpolation * ramp + extrapolation * (1.0 - ramp)
    freqs = np.outer(positions.astype(np.float32), inv_freq)  # (n, 32)
    return (
        (np.cos(freqs) * concentration).astype(np.float32),
        (np.sin(freqs) * concentration).astype(np.float32),
    )


def _host_inputs(x, qkv_w, qkv_b, out_w, out_b, sinks):
    import ml_dtypes

    bf16 = ml_dtypes.bfloat16
    x = np.asarray(x, np.float32)
    qkv_w = np.asarray(qkv_w, np.float32)
    qkv_b = np.asarray(qkv_b, np.float32)
    out_w = np.asarray(out_w, np.float32)
    out_b = np.asarray(out_b, np.float32)
    sinks = np.asarray(sinks, np.float32)

    wq_h = np.ascontiguousarray(
        qkv_w[:2048].reshape(16, 128, NKT, 128).transpose(0, 3, 2, 1).reshape(16, 128, D)
    ).astype(bf16)
    wk_h = np.ascontiguousarray(
        qkv_w[2048:2560].reshape(4, 128, NKT, 128).transpose(0, 3, 2, 1).reshape(4, 128, D)
    ).astype(bf16)
    # wv groups: wv[g][d_local, k2*512 + vf] = Wv[vf, 128*(4g+k2) + d_local]
    wv_h = np.ascontiguousarray(
        qkv_w[2560:3072].T.reshape(4, 4, 128, 512).transpose(0, 2, 1, 3).reshape(4, 128, D)
    ).astype(bf16)
    wo_h = np.ascontiguousarray(out_w.T).reshape(16, 128, D).astype(bf16)
    qbias_h = np.ascontiguousarray(qkv_b[:2048].reshape(16, 128).T)
    kbias_h = np.ascontiguousarray(qkv_b[2048:2560].reshape(4, 128).T)
    vbias_h = qkv_b[2560:3072].reshape(1, 512).astype(bf16)
    outb_h = out_b.reshape(1, D).astype(bf16)
    es = np.exp(sinks).reshape(NQF, 2)          # [head pair, h]
    esinkc_h = np.ascontiguousarray(np.tile(es.T, (4, 1)))  # [8, 16] rows (2qb+h)
    selqb_h = np.zeros((8, NQB, 128), np.float32)
    for qb in range(NQB):
        selqb_h[2 * qb, qb, 0:64] = 1.0
        selqb_h[2 * qb + 1, qb, 64:128] = 1.0
    selqb_h = selqb_h.astype(bf16)
    ones128_h = np.ones((128, 1), bf16)
    onescol_h = np.ones((1, 128), bf16)

    r = np.arange(128)[:, None]
    c = np.arange(128)[None, :]
    mprev = (r > c).astype(np.float32)
    mdiag = (r <= c).astype(np.float32)
    maskt_h = np.concatenate([mprev, mprev, mdiag, mdiag], axis=1).astype(bf16)
    mask0_h0 = np.concatenate(
        [np.zeros((128, 256), np.float32), mdiag, mdiag], axis=1
    ).astype(bf16)

    def sin_tabs(s):
        """Half-shifted signed sin tables for the swap-free rotate-half.
        TA rows [32:64],[96:128] = -s.T (multiplies x2 into out rows 0:32);
        TB rows [0:32],[64:96] = +s.T (multiplies x1 into out rows 32:64)."""
        n = s.shape[0]
        z = np.zeros((32, n), np.float32)
        ta = np.concatenate([z, -s.T, z, -s.T], 0)
        tb = np.concatenate([s.T, z, s.T, z], 0)
        return ta.astype(bf16), tb.astype(bf16)

    in_maps = []
    for core in range(8):
        b, half = core // 2, core % 2
        t0 = half * TQ
        x_pad = np.zeros((TOK, D), np.float32)
        lo = t0 - HALO
        x_pad[max(0, -lo):] = x[b, max(lo, 0) : t0 + TQ]
        xt_h = np.ascontiguousarray(
            x_pad.T.reshape(NKT, 128, TOK).transpose(1, 0, 2)
        ).astype(bf16)
        cq, sq = _rope_tables(np.arange(t0, t0 + TQ))
        ck, sk = _rope_tables(np.clip(np.arange(t0 - HALO, t0 + TQ), 0, None))
        sqa, sqb = sin_tabs(sq)
        ska, skb = sin_tabs(sk)
        in_maps.append(
            {
                "xt": xt_h,
                "wq": wq_h,
                "wk": wk_h,
                "wv": wv_h,
                "wo": wo_h,
                "qbias": qbias_h,
                "kbias": kbias_h,
                "vbias": vbias_h,
                "outb": outb_h,
                "cosq": np.tile(np.concatenate([cq.T, cq.T], 0), (2, 1)).astype(bf16),
                "sinqa": sqa,
                "sinqb": sqb,
                "cosk": np.tile(np.concatenate([ck.T, ck.T], 0), (2, 1)).astype(bf16),
                "sinka": ska,
                "sinkb": skb,
                "maskt": maskt_h,
                "mask0": mask0_h0 if half == 0 else maskt_h,
                "esinkc": esinkc_h,
                "selqb": selqb_h,
                "ones128": ones128_h,
                "onescol": onescol_h,
            }
        )
    return in_maps


def kernel(x, qkv_w, qkv_b, out_w, out_b, sinks, _trace=False, _tmpdir=None):
    nc = _get_nc()
    in_maps = _host_inputs(x, qkv_w, qkv_b, out_w, out_b, sinks)
    kwargs = {}
    if _trace:
        kwargs = dict(trace=True, tmpdir=_tmpdir)
    res = run_bass_kernel_spmd(nc, in_maps, core_ids=list(range(8)), **kwargs)
    out = np.empty((B, T, D), np.float32)
    for core in range(8):
        b, half = core // 2, core % 2
        out[b, half * TQ : half * TQ + TQ] = res.results[core]["y"]
    if _trace:
        kernel._last_results = res
    return out


# revision 55
# speedup vs baseline: 1.0135x; 1.0135x over previous
"""Trainium2 Bass kernel for nn_AttentionBlock (sliding-window GQA, gpt-oss style).

Sharding: pure data-parallel over tokens. B=4 batches x 2 T-halves of 512
tokens = 8 shards, one per NeuronCore; the 128-token sliding window means each
shard only needs a 128-token K/V halo, so there is no cross-core traffic.

v2 (this file) vs the v1 baseline:
  * every matmul operand is bf16 (f32 PSUM accumulate) -> half the weight DMA
    and full-rate PE everywhere, incl. the small attention tiles.
  * attention uses 128-query x two-128-key triangle blocks (prev + diag).
    Per (head-pair, 128-query block) the scores psum is [128k, (kb, h, q)] and
    the visible set is exactly the two complementary triangles, applied as a
    multiplicative bf16 0/1 mask on the exp'd tile (DVE), so no PE cycles are
    burned on additive mask matmuls.
  * PV runs per head with tile_position=(0,0)/(0,64) so the two heads' O.T
    land stacked in one [128=(2h x 64vf), 128q] psum tile -- which is exactly
    the lhsT layout the out-projection wants; the normalize multiply is the
    evacuation (one DVE op per unit).
  * softmax denominators: ones-column matmuls into a [1, (h,q)] psum row,
    DMA-gathered 4 rows at a time into a per-head-pair [8, 128] tile, one
    reciprocal_approx_fast per head-pair, then broadcast back to 64 vf
    partitions with a tiny selector matmul ([8,128] x [8,128] -> [128,128]).
  * per-head attention sinks are folded in as a per-partition tensor_scalar
    add on the packed denominator tile.

On-chip layouts (per core):
  xT        [dmodel, 640tok] bf16 (halo 128 + 512 own; halo zero-padded on
                                   the first half of each sequence)
  K.T       [64, 640] bf16 per kv head (rotated)
  Q.T       [64, 2, 512] bf16 per head pair (rotated)
  V         [128tok, 8kv, 64] bf16 per 128-token chunk (direct [tok, feat]
            projection: lhsT = xT tile, rhs = wv.T tile -- no PE transpose)
  scores    S.T [128key, (kb, h, 128q)] psum; exp on scalar engine; 0/1
            triangle mask on DVE
  out proj  Y [tok, dmodel] accumulated over 16 feature chunks + K=1 bias
            matmul, evacuated f32
"""

import math
from contextlib import ExitStack

import numpy as np

import concourse.bacc as bacc
import concourse.tile as tile
from concourse import mybir
from concourse.bass_utils import run_bass_kernel_spmd

_DEBUG = False
F32 = mybir.dt.float32
BF16 = mybir.dt.bfloat16
AF = mybir.ActivationFunctionType
ALU = mybir.AluOpType

B, T, D = 4, 1024, 2048
HEAD_DIM = 64
N_HEADS = 32
N_KV = 8
WINDOW = 128
SM_SCALE = 1.0 / math.sqrt(HEAD_DIM)
ROPE_THETA = 150000.0
SCALING = 32.0
NTK_ALPHA = 1.0
NTK_BETA = 32.0
ICL = 1024

TQ = 512          # queries per shard
HALO = 128
TOK = TQ + HALO   # 640 tokens of K/V context per shard
NKT = D // 128    # 16 contraction tiles over dmodel
NQF = 16          # Q feature tiles (2048 features = 16 head pairs)
NKF = 4           # K feature tiles (512 features = 4 kv-head pairs)
NVC = TOK // 128  # 5 V token chunks
NQB = TQ // 128   # 4 query blocks per shard


def _build_nc():
    nc = bacc.Bacc("TRN2", target_bir_lowering=False, debug=False)

    # ---- DRAM I/O ----
    # xt is partition-major: one DMA, 128 contiguous ~20KB descriptors
    xt = nc.dram_tensor("xt", (128, NKT, TOK), BF16, kind="ExternalInput")
    wq = nc.dram_tensor("wq", (NQF, 128, D), BF16, kind="ExternalInput")
    wk = nc.dram_tensor("wk", (NKF, 128, D), BF16, kind="ExternalInput")
    wv = nc.dram_tensor("wv", (4, 128, D), BF16, kind="ExternalInput")
    wo = nc.dram_tensor("wo", (16, 128, D), BF16, kind="ExternalInput")
    qbias = nc.dram_tensor("qbias", (128, NQF), F32, kind="ExternalInput")
    kbias = nc.dram_tensor("kbias", (128, NKF), F32, kind="ExternalInput")
    vbias = nc.dram_tensor("vbias", (1, 512), BF16, kind="ExternalInput")
    outb = nc.dram_tensor("outb", (1, D), BF16, kind="ExternalInput")
    # rope tables: cos duplicated per head; sinA/sinB are the half-shifted
    # signed sin tables so the rotate-half multiply needs no partition swap
    cosq = nc.dram_tensor("cosq", (128, TQ), BF16, kind="ExternalInput")
    sinqa = nc.dram_tensor("sinqa", (128, TQ), BF16, kind="ExternalInput")
    sinqb = nc.dram_tensor("sinqb", (128, TQ), BF16, kind="ExternalInput")
    cosk = nc.dram_tensor("cosk", (128, TOK), BF16, kind="ExternalInput")
    sinka = nc.dram_tensor("sinka", (128, TOK), BF16, kind="ExternalInput")
    sinkb = nc.dram_tensor("sinkb", (128, TOK), BF16, kind="ExternalInput")
    maskt = nc.dram_tensor("maskt", (128, 512), BF16, kind="ExternalInput")
    mask0 = nc.dram_tensor("mask0", (128, 512), BF16, kind="ExternalInput")
    esinkc = nc.dram_tensor("esinkc", (8, NQF), F32, kind="ExternalInput")
    selqb = nc.dram_tensor("selqb", (8, NQB, 128), BF16, kind="ExternalInput")
    ones128 = nc.dram_tensor("ones128", (128, 1), BF16, kind="ExternalInput")
    onescol = nc.dram_tensor("onescol", (1, 128), BF16, kind="ExternalInput")
    y = nc.dram_tensor("y", (TQ, D), F32, kind="ExternalOutput")
    if _DEBUG:
        dbg_k = nc.dram_tensor("dbg_k", (64, TOK), BF16, kind="ExternalOutput")
        dbg_q = nc.dram_tensor("dbg_q", (64, 2, TQ), BF16, kind="ExternalOutput")
        dbg_v = nc.dram_tensor("dbg_v", (128, 64), BF16, kind="ExternalOutput")
        dbg_pt = nc.dram_tensor("dbg_pt", (128, 512), BF16, kind="ExternalOutput")
        dbg_dn = nc.dram_tensor("dbg_dn", (8, 128), F32, kind="ExternalOutput")
        dbg_opk = nc.dram_tensor("dbg_opk", (128, 128), BF16, kind="ExternalOutput")

    with tile.TileContext(nc) as tc, ExitStack() as ctx:
        ep = ctx.enter_context
        const = ep(tc.tile_pool(name="const", bufs=1))
        wqp = ep(tc.tile_pool(name="wqp", bufs=4))
        kthp = ep(tc.tile_pool(name="kthp", bufs=8))    # rotated K per kv head
        vsbp = ep(tc.tile_pool(name="vsbp", bufs=NVC))  # V chunks, held
        qhdp = ep(tc.tile_pool(name="qhdp", bufs=3))
        qrbp = ep(tc.tile_pool(name="qrbp", bufs=3))
        ptbep = ep(tc.tile_pool(name="ptbep", bufs=2))
        ptbp = ep(tc.tile_pool(name="ptbp", bufs=4))
        prp = ep(tc.tile_pool(name="prp", bufs=2))
        pwp = ep(tc.tile_pool(name="pwp", bufs=2))
        opknp = ep(tc.tile_pool(name="opknp", bufs=NQF * NQB))  # held for out proj
        drpp = ep(tc.tile_pool(name="drpp", bufs=2))
        dnsp = ep(tc.tile_pool(name="dnsp", bufs=2))
        rpssp = ep(tc.tile_pool(name="rpssp", bufs=2))
        drsp = ep(tc.tile_pool(name="drsp", bufs=2))
        drrp = ep(tc.tile_pool(name="drrp", bufs=2))
        drbp = ep(tc.tile_pool(name="drbp", bufs=2))
        xtp = ep(tc.tile_pool(name="xtp", bufs=4))
        # K/V-phase-only pools: created last (top of the SBUF pool stack) so
        # they can be released before the wo prefetch reuses their space
        kvctx = ExitStack()
        wkp = kvctx.enter_context(tc.tile_pool(name="wkp", bufs=4))
        wvp = kvctx.enter_context(tc.tile_pool(name="wvp", bufs=4))
        khdp = kvctx.enter_context(tc.tile_pool(name="khdp", bufs=2))
        if True:
            # ---- highest-priority input DMAs first, spread over queues:
            # xt as 4 parallel transfers (2 on gpsimd, 2 on vector), K weights
            # ahead of Q weights on sync so the K phase is never starved
            xtq = []
            for g in range(4):
                t = xtp.tile([128, 4, TOK], BF16, tag="xt", name=f"xt{g}")
                eng = nc.gpsimd if g % 2 == 0 else nc.scalar
                eng.dma_start(out=t, in_=xt[:, 4 * g : 4 * g + 4, :])
                xtq.append(t)
            xts = [xtq[kt // 4][:, kt % 4, :] for kt in range(NKT)]
            wks = []
            for fk in range(NKF):
                t = wkp.tile([128, NKT, 128], BF16, tag="wk")
                nc.sync.dma_start(out=t, in_=wk[fk])
                wks.append(t)
            wvs = []
            for g in range(4):
                t = wvp.tile([128, 4, 512], BF16, tag="wv")
                eng = nc.sync if g < 2 else nc.gpsimd
                eng.dma_start(out=t, in_=wv[g].rearrange("p (g f) -> p g f", g=4))
                wvs.append(t)
            # first few Q weight tiles on sync (rest just-in-time in the loop)
            wqs = [None] * NQF
            for qf in range(4):
                wqs[qf] = wqp.tile([128, NKT, 128], BF16, tag="wq", name=f"wq{qf}")
                nc.sync.dma_start(out=wqs[qf], in_=wq[qf])

            # ---- constants (scalar queue, K-phase needs first) ----
            kbias_sb = const.tile([128, NKF], F32)
            nc.scalar.dma_start(out=kbias_sb, in_=kbias[:])
            cosk_sb = const.tile([128, TOK], BF16)
            nc.scalar.dma_start(out=cosk_sb, in_=cosk[:])
            sinka_sb = const.tile([128, TOK], BF16)
            nc.scalar.dma_start(out=sinka_sb, in_=sinka[:])
            sinkb_sb = const.tile([128, TOK], BF16)
            nc.scalar.dma_start(out=sinkb_sb, in_=sinkb[:])
            vbias_sb = const.tile([1, 512], BF16)
            nc.scalar.dma_start(out=vbias_sb, in_=vbias[:])
            onescol_sb = const.tile([1, 128], BF16)
            nc.scalar.dma_start(out=onescol_sb, in_=onescol[:])
            qbias_sb = const.tile([128, NQF], F32)
            nc.scalar.dma_start(out=qbias_sb, in_=qbias[:])
            cosq_sb = const.tile([128, TQ], BF16)
            nc.scalar.dma_start(out=cosq_sb, in_=cosq[:])
            sinqa_sb = const.tile([128, TQ], BF16)
            nc.scalar.dma_start(out=sinqa_sb, in_=sinqa[:])
            sinqb_sb = const.tile([128, TQ], BF16)
            nc.scalar.dma_start(out=sinqb_sb, in_=sinqb[:])
            maskt_sb = const.tile([128, 512], BF16)
            nc.scalar.dma_start(out=maskt_sb, in_=maskt[:])
            mask0_sb = const.tile([128, 512], BF16)
            nc.scalar.dma_start(out=mask0_sb, in_=mask0[:])
            esinkc_sb = const.tile([8, NQF], F32)
            nc.scalar.dma_start(out=esinkc_sb, in_=esinkc[:])
            selqb_sb = const.tile([8, NQB, 128], BF16)
            nc.scalar.dma_start(out=selqb_sb, in_=selqb[:])
            ones128_sb = const.tile([128, 1], BF16)
            nc.scalar.dma_start(out=ones128_sb, in_=ones128[:])
            outb_sb = const.tile([1, D], BF16)
            nc.scalar.dma_start(out=outb_sb, in_=outb[:])

            pctx = ExitStack()
            pspj = pctx.enter_context(tc.tile_pool(name="pspj", bufs=2, space="PSUM"))
            psst = pctx.enter_context(tc.tile_pool(name="psst", bufs=2, space="PSUM"))
            psop = pctx.enter_context(tc.tile_pool(name="psop", bufs=2, space="PSUM"))
            psrp = pctx.enter_context(tc.tile_pool(name="psrp", bufs=1, space="PSUM"))
            psdn = pctx.enter_context(tc.tile_pool(name="psdn", bufs=1, space="PSUM"))
            if True:
                def rope_pair(dsts, src, cos_t, sina_t, sinb_t, n):
                    """Rotate both 64-row heads of src [128, n] into dsts[h]
                    [64, n] without any partition swap: the rotate-half
                    product uses half-shifted sin tables so every
                    tensor_tensor's two inputs share a base partition."""
                    pr = prp.tile([128, TOK], BF16, tag="ropepr")
                    nc.vector.tensor_mul(
                        out=pr[:, :n], in0=src[:, :n], in1=cos_t[:, :n]
                    )
                    pw = pwp.tile([128, TOK], BF16, tag="ropepw")
                    for b0 in (0, 64):
                        nc.vector.tensor_mul(
                            out=pw[b0 : b0 + 32, :n],
                            in0=src[b0 + 32 : b0 + 64, :n],
                            in1=sina_t[b0 + 32 : b0 + 64, :n],
                        )
                        nc.vector.tensor_mul(
                            out=pw[b0 + 32 : b0 + 64, :n],
                            in0=src[b0 : b0 + 32, :n],
                            in1=sinb_t[b0 : b0 + 32, :n],
                        )
                    for h2 in range(2):
                        nc.vector.tensor_add(
                            out=dsts[h2],
                            in0=pr[64 * h2 : 64 * h2 + 64, :n],
                            in1=pw[64 * h2 : 64 * h2 + 64, :n],
                        )

                # ---- K projection + rope: 8 kv heads as 4 pairs ----
                # contraction permuted to xt-quarter arrival order
                # (gpsimd: q0, q2; scalar: q1, q3) so the first chains
                # never stall on a late quarter
                kperm = [0, 1, 2, 3, 8, 9, 10, 11, 4, 5, 6, 7, 12, 13, 14, 15]
                kth = [None] * N_KV
                for fk in range(NKF):
                    wsb = wks[fk]
                    khd = khdp.tile([128, TOK], BF16, tag="khd")
                    for c0 in (0, 320):
                        pst = pspj.tile([128, 512], F32, tag="pj")
                        for ki, k in enumerate(kperm):
                            nc.tensor.matmul(
                                pst[:, 0:320],
                                wsb[:, k, :],
                                xts[k][:, c0 : c0 + 320],
                                start=(ki == 0),
                                stop=(ki == NKT - 1),
                            )
                        nc.scalar.activation(
                            out=khd[:, c0 : c0 + 320],
                            in_=pst[:, 0:320],
                            func=AF.Identity,
                            bias=kbias_sb[:, fk : fk + 1],
                        )
                    kt0 = kthp.tile([64, TOK], BF16, tag="kth")
                    kt1 = kthp.tile([64, TOK], BF16, tag="kth")
                    rope_pair([kt0, kt1], khd, cosk_sb, sinka_sb, sinkb_sb, TOK)
                    kth[2 * fk] = kt0
                    kth[2 * fk + 1] = kt1
                if _DEBUG:
                    nc.sync.dma_start(out=dbg_k[:], in_=kth[0])

                # ---- V: direct [tok, feat] projection ----
                vsbs = []
                for c in range(NVC):
                    psv = pspj.tile([128, 512], F32, tag="pj")
                    for k in range(NKT):
                        nc.tensor.matmul(
                            psv,
                            xts[k][:, 128 * c : 128 * c + 128],
                            wvs[k // 4][:, k % 4, :],
                            start=(k == 0),
                            stop=False,
                        )
                    nc.tensor.matmul(
                        psv, onescol_sb, vbias_sb, start=False, stop=True
                    )
                    vsb = vsbp.tile([128, N_KV, 64], BF16, tag="v")
                    nc.scalar.activation(
                        out=vsb,
                        in_=psv.rearrange("p (h d) -> p h d", h=N_KV),
                        func=AF.Copy,
                    )
                    vsbs.append(vsb)
                if _DEBUG:
                    nc.sync.dma_start(out=dbg_v[:], in_=vsbs[2][:, 0, :])

                # ---- out-proj weight prefetch (first half) ----
                kvctx.close()  # free wk/wv/khd SBUF for the wo tiles
                wop = ctx.enter_context(tc.tile_pool(name="wop", bufs=32))
                wos = []
                for ft in range(16):
                    wosb = wop.tile([128, 1024], BF16, tag="wo")
                    nc.gpsimd.dma_start(out=wosb, in_=wo[ft][:, 0:1024])
                    wos.append(wosb)

                # ---- Q projection + attention per head pair ----
                opkns = [[None] * NQB for _ in range(NQF)]
                for qf in range(NQF):
                    h_kv = qf // 2
                    if qf + 4 < NQF:
                        wqs[qf + 4] = wqp.tile([128, NKT, 128], BF16, tag="wq", name=f"wq{qf+4}")
                        nc.sync.dma_start(out=wqs[qf + 4], in_=wq[qf + 4])
                    if qf == 11:
                        # prefetch second half of out-proj weights
                        for ft in range(16):
                            wosb = wop.tile([128, 1024], BF16, tag="wo")
                            nc.gpsimd.dma_start(
                                out=wosb, in_=wo[ft][:, 1024:2048]
                            )
                            wos.append(wosb)
                    wsb = wqs[qf]
                    psq = pspj.tile([128, 512], F32, tag="pj")
                    for k in range(NKT):
                        nc.tensor.matmul(
                            psq,
                            wsb[:, k, :],
                            xts[k][:, HALO:TOK],
                            start=(k == 0),
                            stop=(k == NKT - 1),
                        )
                    qhd = qhdp.tile([128, TQ], BF16, tag="qhd")
                    nc.scalar.activation(
                        out=qhd, in_=psq, func=AF.Identity,
                        bias=qbias_sb[:, qf : qf + 1],
                    )
                    qrb = qrbp.tile([64, 2, TQ], BF16, tag="qrb")
                    rope_pair(
                        [qrb[:, 0, :], qrb[:, 1, :]], qhd,
                        cosq_sb, sinqa_sb, sinqb_sb, TQ,
                    )
                    if _DEBUG and qf == 0:
                        nc.sync.dma_start(out=dbg_q[:], in_=qrb)

                    dn = psdn.tile([1, 512], F32, tag="dn")
                    drp = drpp.tile([8, 128], F32, tag="drp")
                    ops4 = psop.tile([128, NQB, 128], F32, tag="ops")
                    for qb in range(NQB):
                        # scores: prev + diag key blocks
                        st = psst.tile([128, 512], F32, tag="st")
                        qs = qrb[:, :, 128 * qb : 128 * qb + 128]
                        nc.tensor.matmul(
                            st[:, 0:256],
                            kth[h_kv][:, 128 * qb : 128 * qb + 128],
                            qs, start=True, stop=True,
                        )
                        nc.tensor.matmul(
                            st[:, 256:512],
                            kth[h_kv][:, 128 * qb + 128 : 128 * qb + 256],
                            qs, start=True, stop=True,
                        )
                        ptbe = ptbep.tile([128, 512], BF16, tag="ptbe")
                        nc.scalar.activation(
                            out=ptbe, in_=st, func=AF.Exp, scale=SM_SCALE
                        )
                        ptb = ptbp.tile([128, 512], BF16, tag="ptb")
                        nc.vector.tensor_mul(
                            out=ptb, in0=ptbe,
                            in1=(mask0_sb if qb == 0 else maskt_sb),
                        )
                        if _DEBUG and qf == 0 and qb == 1:
                            nc.sync.dma_start(out=dbg_pt[:], in_=ptb)
                        # denominator rows: [1, (h, q)] accumulated over kb
                        dslot = dn[0:1, 256 * (qb % 2) : 256 * (qb % 2) + 256]
                        nc.tensor.matmul(
                            dslot, ones128_sb, ptb[:, 0:256], start=True, stop=False
                        )
                        nc.tensor.matmul(
                            dslot, ones128_sb, ptb[:, 256:512], start=False, stop=True
                        )
                        # PV: per head, stacked halves of one [128, 128] slot
                        for h2 in range(2):
                            nc.tensor.matmul(
                                ops4[64 * h2 : 64 * h2 + 64, qb, :],
                                vsbs[qb][:, h_kv, :],
                                ptb[:, 128 * h2 : 128 * h2 + 128],
                                start=True, stop=False,
                                tile_position=(0, 64 * h2),
                            )
                            nc.tensor.matmul(
                                ops4[64 * h2 : 64 * h2 + 64, qb, :],
                                vsbs[qb + 1][:, h_kv, :],
                                ptb[:, 256 + 128 * h2 : 256 + 128 * h2 + 128],
                                start=False, stop=True,
                                tile_position=(0, 64 * h2),
                            )
                        if qb % 2 == 1:
                            # evac both units' denom rows, gather -> [4, 128]
                            dn_sb = dnsp.tile([1, 512], F32, tag="dnsb")
                            nc.scalar.activation(out=dn_sb, in_=dn, func=AF.Copy)
                            nc.scalar.dma_start(
                                out=drp[2 * (qb - 1) : 2 * (qb - 1) + 4, :],
                                in_=dn_sb,
                            )
                    # packed sink-add + reciprocal for this head pair
                    drs = drsp.tile([8, 128], F32, tag="drs")
                    nc.vector.tensor_scalar(
                        out=drs, in0=drp,
                        scalar1=esinkc_sb[:, qf : qf + 1], scalar2=None,
                        op0=ALU.add,
                    )
                    drr = drrp.tile([8, 128], F32, tag="drr")
                    nc.vector.reciprocal_approx_fast(out=drr, in_=drs)
                    drb = drbp.tile([8, 128], BF16, tag="drb")
                    nc.vector.tensor_copy(out=drb, in_=drr)
                    if _DEBUG and qf == 0:
                        nc.sync.dma_start(out=dbg_dn[:], in_=drr)
                    rps4 = psrp.tile([128, NQB, 128], F32, tag="rps")
                    for qb in range(NQB):
                        nc.tensor.matmul(
                            rps4[:, qb, :], selqb_sb[:, qb, :], drb,
                            start=True, stop=True,
                        )
                    # DVE may read only one PSUM input: evac rps first
                    rps_sb = rpssp.tile([128, NQB, 128], BF16, tag="rpssb")
                    nc.scalar.activation(out=rps_sb, in_=rps4, func=AF.Copy)
                    for qb in range(NQB):
                        opkn = opknp.tile([128, 128], BF16, tag="opkn")
                        nc.vector.tensor_mul(
                            out=opkn, in0=ops4[:, qb, :], in1=rps_sb[:, qb, :]
                        )
                        opkns[qf][qb] = opkn
                        if _DEBUG and qf == 0 and qb == 1:
                            nc.sync.dma_start(out=dbg_opk[:], in_=opkn)

            # ---- output projection ----
            pctx.close()
            yp = ctx.enter_context(tc.tile_pool(name="yp", bufs=2))
            psy = ctx.enter_context(tc.tile_pool(name="psy", bufs=2, space="PSUM"))
            if True:
                for chp in range(2):
                    for qb in range(NQB):
                        for c2 in range(2):
                            ch = chp * 2 + c2
                            yps = psy.tile([128, 512], F32, tag="psy")
                            for ft in range(16):
                                nc.tensor.matmul(
                                    yps,
                                    opkns[ft][qb],
                                    wos[chp * 16 + ft][:, c2 * 512 : c2 * 512 + 512],
                                    start=(ft == 0),
                                    stop=False,
                                )
                            nc.tensor.matmul(
                                yps,
                                onescol_sb,
                                outb_sb[:, ch * 512 : ch * 512 + 512],
                                start=False, stop=True,
                            )
                            ysb = yp.tile([128, 512], F32, tag="y")
                            nc.scalar.activation(out=ysb, in_=yps, func=AF.Copy)
                            nc.sync.dma_start(
                                out=y[qb * 128 : (qb + 1) * 128, ch * 512 : ch * 512 + 512],
                                in_=ysb,
                            )

    nc.compile()
    return nc


_NC_CACHE = None


def _get_nc():
    global _NC_CACHE
    if _NC_CACHE is None:
        _NC_CACHE = _build_nc()
    return _NC_CACHE


def _rope_tables(positions):
    """fp32 YaRN/NTK-by-parts tables, matching the reference bit-for-bit."""
    d_half = HEAD_DIM // 2
    freq = ROPE_THETA ** (np.arange(0, HEAD_DIM, 2, dtype=np.float32) / HEAD_DIM)
    concentration = 0.1 * math.log(SCALING) + 1.0
    low = d_half * math.log(ICL / (NTK_BETA * 2 * math.pi)) / math.log(ROPE_THETA)
    high = d_half * math.log(ICL / (NTK_ALPHA * 2 * math.pi)) / math.log(ROPE_THETA)
    interpolation = 1.0 / (SCALING * freq)
    extrapolation = 1.0 / freq
    ramp = np.clip(
        (np.arange(d_half, dtype=np.float32) - low) / (high - low), 0.0, 1.0
    )
    inv_freq = interpolation * ramp + extrapolation * (1.0 - ramp)
    freqs = np.outer(positions.astype(np.float32), inv_freq)  # (n, 32)
    return (
        (np.cos(freqs) * concentration).astype(np.float32),
        (np.sin(freqs) * concentration).astype(np.float32),
    )


def _host_inputs(x, qkv_w, qkv_b, out_w, out_b, sinks):
    import ml_dtypes

    bf16 = ml_dtypes.bfloat16
    x = np.asarray(x, np.float32)
    qkv_w = np.asarray(qkv_w, np.float32)
    qkv_b = np.asarray(qkv_b, np.float32)
    out_w = np.asarray(out_w, np.float32)
    out_b = np.asarray(out_b, np.float32)
    sinks = np.asarray(sinks, np.float32)

    wq_h = np.ascontiguousarray(
        qkv_w[:2048].reshape(16, 128, NKT, 128).transpose(0, 3, 2, 1).reshape(16, 128, D)
    ).astype(bf16)
    wk_h = np.ascontiguousarray(
        qkv_w[2048:2560].reshape(4, 128, NKT, 128).transpose(0, 3, 2, 1).reshape(4, 128, D)
    ).astype(bf16)
    # wv groups: wv[g][d_local, k2*512 + vf] = Wv[vf, 128*(4g+k2) + d_local]
    wv_h = np.ascontiguousarray(
        qkv_w[2560:3072].T.reshape(4, 4, 128, 512).transpose(0, 2, 1, 3).reshape(4, 128, D)
    ).astype(bf16)
    wo_h = np.ascontiguousarray(out_w.T).reshape(16, 128, D).astype(bf16)
    qbias_h = np.ascontiguousarray(qkv_b[:2048].reshape(16, 128).T)
    kbias_h = np.ascontiguousarray(qkv_b[2048:2560].reshape(4, 128).T)
    vbias_h = qkv_b[2560:3072].reshape(1, 512).astype(bf16)
    outb_h = out_b.reshape(1, D).astype(bf16)
    es = np.exp(sinks).reshape(NQF, 2)          # [head pair, h]
    esinkc_h = np.ascontiguousarray(np.tile(es.T, (4, 1)))  # [8, 16] rows (2qb+h)
    selqb_h = np.zeros((8, NQB, 128), np.float32)
    for qb in range(NQB):
        selqb_h[2 * qb, qb, 0:64] = 1.0
        selqb_h[2 * qb + 1, qb, 64:128] = 1.0
    selqb_h = selqb_h.astype(bf16)
    ones128_h = np.ones((128, 1), bf16)
    onescol_h = np.ones((1, 128), bf16)

    r = np.arange(128)[:, None]
    c = np.arange(128)[None, :]
    mprev = (r > c).astype(np.float32)
    mdiag = (r <= c).astype(np.float32)
    maskt_h = np.concatenate([mprev, mprev, mdiag, mdiag], axis=1).astype(bf16)
    mask0_h0 = np.concatenate(
        [np.zeros((128, 256), np.float32), mdiag, mdiag], axis=1
    ).astype(bf16)

    def sin_tabs(s):
        """Half-shifted signed sin tables for the swap-free rotate-half.
        TA rows [32:64],[96:128] = -s.T (multiplies x2 into out rows 0:32);
        TB rows [0:32],[64:96] = +s.T (multiplies x1 into out rows 32:64)."""
        n = s.shape[0]
        z = np.zeros((32, n), np.float32)
        ta = np.concatenate([z, -s.T, z, -s.T], 0)
        tb = np.concatenate([s.T, z, s.T, z], 0)
        return ta.astype(bf16), tb.astype(bf16)

    in_maps = []
    for core in range(8):
        b, half = core // 2, core % 2
        t0 = half * TQ
        x_pad = np.zeros((TOK, D), np.float32)
        lo = t0 - HALO
        x_pad[max(0, -lo):] = x[b, max(lo, 0) : t0 + TQ]
        xt_h = np.ascontiguousarray(
            x_pad.T.reshape(NKT, 128, TOK).transpose(1, 0, 2)
        ).astype(bf16)
        cq, sq = _rope_tables(np.arange(t0, t0 + TQ))
        ck, sk = _rope_tables(np.clip(np.arange(t0 - HALO, t0 + TQ), 0, None))
        sqa, sqb = sin_tabs(sq)
        ska, skb = sin_tabs(sk)
        in_maps.append(
            {
                "xt": xt_h,
                "wq": wq_h,
                "wk": wk_h,
                "wv": wv_h,
                "wo": wo_h,
                "qbias": qbias_h,
                "kbias": kbias_h,
                "vbias": vbias_h,
                "outb": outb_h,
                "cosq": np.tile(np.concatenate([cq.T, cq.T], 0), (2, 1)).astype(bf16),
                "sinqa": sqa,
                "sinqb": sqb,
                "cosk": np.tile(np.concatenate([ck.T, ck.T], 0), (2, 1)).astype(bf16),
                "sinka": ska,
                "sinkb": skb,
                "maskt": maskt_h,
                "mask0": mask0_h0 if half == 0 else maskt_h,
                "esinkc": esinkc_h,
                "selqb": selqb_h,
                "ones128": ones128_h,
                "onescol": onescol_h,
            }
        )
    return in_maps


def kernel(x, qkv_w, qkv_b, out_w, out_b, sinks, _trace=False, _tmpdir=None):
    nc = _get_nc()
    in_maps = _host_inputs(x, qkv_w, qkv_b, out_w, out_b, sinks)
    kwargs = {}
    if _trace:
        kwargs = dict(trace=True, tmpdir=_tmpdir)
    res = run_bass_kernel_spmd(nc, in_maps, core_ids=list(range(8)), **kwargs)
    out = np.empty((B, T, D), np.float32)
    for core in range(8):
        b, half = core // 2, core % 2
        out[b, half * TQ : half * TQ + TQ] = res.results[core]["y"]
    if _trace:
        kernel._last_results = res
    return out
